# revision 17
# baseline (speedup 1.0000x reference)
# Involution2d (K=7) Trainium2 kernel — 8-core SPMD, batch+spatial sharding.
#
# Sharding: 8 cores = (batch b in 0..3) x (H-half in 0..1). Each core computes
# a [128, 32, 64] output block.
#
# Per-core pipeline (fp16 data path, rel-err budget 2e-2):
#   1. f = relu(w1s^T @ xd + b1f)           TensorE + ScalarE   [32, 2048]
#   2. per offset o (49 total):
#      bc_o  = W2BC_o @ f                   TensorE (K=32)      [128, 2048] PSUM
#        (W2BC_o = w2 row o replicated into 128 columns -> the per-pixel
#         kernel value arrives already broadcast over channels; kerm is
#         never materialized and no per-offset DMA is needed)
#      bcs_o = bc_o + b2[o]                 ScalarE PSUM->SBUF fp16 (2 halves)
#      prod  = bcs_o * x_shift(o)           VectorE fp16 2x-mode
#      acc  += prod                         VectorE fp16 2x-mode
#
# x lives in a guarded stride-70 row layout (3 zero cols each side, 3 halo
# rows top/bottom) so every shifted read is exact zero padding — no mask.
# Shifted reads use 3D APs [128, 32, 64]; a twin copy of x offset by one
# element keeps odd-dj offsets 4-byte aligned (DVE 2x_1P mode requirement).
# acc/bcs/prod/f are dense [.., 2048] stride-1 tiles.
import numpy as np

EPS = 1e-5
KK = 7
C = 128
H = 64
W = 64
B = 4
HH = 32              # output rows per core
XROW = 70            # padded row stride: 3 | 64 | 3
NH = HH + 6          # rows incl. 3-row halos
XPAD = 4             # edge guard (even: preserves dj parity)
NXF = NH * XROW + 2 * XPAD   # 2668 x columns per core
QOFF = XPAD + 3 * XROW       # start of own rows in guarded coords (214)
P = HH * W           # 2048 dense output pixels
MMCH = 512           # matmul moving chunk (= PSUM bank, fp32)
GP_OFFS = (8, 15, 22, 29, 36, 43)   # offsets on GpSimd (fp32 path)

_STATE = {}


def _build():
    import concourse.tile as tile
    from concourse import bacc, mybir

    f32 = mybir.dt.float32
    f16 = mybir.dt.float16
    nc = bacc.Bacc("TRN2", target_bir_lowering=False, debug=False)

    xa_d = nc.dram_tensor("xa", [C, NXF], f16, kind="ExternalInput").ap()
    w1sT_d = nc.dram_tensor("w1sT", [C, 32], f16, kind="ExternalInput").ap()
    b1f_d = nc.dram_tensor("b1f", [32, 1], f32, kind="ExternalInput").ap()
    w2bc_d = nc.dram_tensor("w2bc", [32, 49 * C], f16, kind="ExternalInput").ap()
    b2bc_d = nc.dram_tensor("b2bc", [C, 49], f32, kind="ExternalInput").ap()
    out_d = nc.dram_tensor("out", [C, P], f16, kind="ExternalOutput").ap()

    with tile.TileContext(nc) as tc:
        with (
            tc.tile_pool(name="consts", bufs=1) as cpool,
            tc.tile_pool(name="bcs", bufs=4) as spool,
            tc.tile_pool(name="prod", bufs=3) as ppool,
            tc.tile_pool(name="gbcs", bufs=2) as gbpool,
            tc.tile_pool(name="gprod", bufs=2) as gppool,
            tc.tile_pool(name="pbc", bufs=4, space="PSUM") as pbc,
        ):
            # spread input DMAs across engines -> parallel hardware queues,
            # ordered so each consumer's gate arrives as late as it is needed
            HP = P // 2
            # xa: rows 0..18 on sync, rows 19..37 on scalar queue, first
            XSPL = XPAD + 19 * XROW
            xa = cpool.tile([C, NXF], f16, tag="xa")
            nc.sync.dma_start(xa[:, :XSPL], xa_d[:, :XSPL])
            nc.scalar.dma_start(xa[:, XSPL:], xa_d[:, XSPL:])
            w1sT = cpool.tile([C, 32], f16, tag="w1")
            nc.sync.dma_start(w1sT[:], w1sT_d)
            b1f = cpool.tile([32, 1], f32, tag="b1")
            nc.sync.dma_start(b1f[:], b1f_d)
            b2bc = cpool.tile([C, 49], f32, tag="b2bc")
            nc.sync.dma_start(b2bc[:], b2bc_d)
            w2bc = cpool.tile([32, 49 * C], f16, tag="w2bc")
            nc.scalar.dma_start(w2bc[:, :8 * C], w2bc_d[:, :8 * C])
            nc.gpsimd.dma_start(w2bc[:, 8 * C:], w2bc_d[:, 8 * C:])
            # twin of xa shifted by one element (keeps odd-dj reads 4B-aligned)
            xb = cpool.tile([C, NXF], f16, tag="xb")

            f_sb = cpool.tile([32, P], f16, tag="f")
            acc = cpool.tile([C, P], f16, tag="acc")
            acc_gp = cpool.tile([C, P], f32, tag="accgp")
            xaf = cpool.tile([C, NXF], f32, tag="xaf")

            # guarded-layout shifted views of x (3D: [128, 32 rows, 64 w])
            xar = xa[:, XPAD:XPAD + NH * XROW].rearrange("p (h w) -> p h w", w=XROW)
            xbr = xb[:, XPAD:XPAD + NH * XROW].rearrange("p (h w) -> p h w", w=XROW)

            # ---- kernel-feature generation: f = relu(w1s^T @ xd + b1f) ----
            nc.gpsimd.tensor_copy(xb[:, :NXF - 1], xa[:, 1:])
            nc.gpsimd.tensor_copy(xaf[:], xa[:])
            xfr = xaf[:, XPAD:XPAD + NH * XROW].rearrange(
                "p (h w) -> p h w", w=XROW)
            for hg in range(2):
                f_ps = pbc.tile([32, P // 2], f32, tag="bc")
                for q in range(2):
                    r = 3 + 8 * (2 * hg + q)
                    xmv = xar[:, r:r + 8, 3:67]
                    nc.tensor.matmul(
                        f_ps[:, q * MMCH:(q + 1) * MMCH], w1sT[:],
                        xmv, start=True, stop=True,
                    )
                nc.scalar.activation(
                    f_sb[:, hg * HP:(hg + 1) * HP], f_ps[:],
                    mybir.ActivationFunctionType.Relu, bias=b1f[:],
                )

            # ---- involution accumulate over the 49 offsets ----
            HB = P // 2  # 1024: evac half (PSUM tile = 2 banks)
            for o in range(49):
                ip, jp = divmod(o, 7)
                di, dj = ip - 3, jp - 3
                gp = o in GP_OFFS
                if gp:
                    bcs = gbpool.tile([C, P], f32, tag="gbcs")
                else:
                    bcs = spool.tile([C, P], f16, tag="bcs")
                for h2 in range(2):
                    bc = pbc.tile([C, HB], f32, tag="bc")
                    for ci in range(HB // MMCH):
                        c0 = h2 * HB + ci * MMCH
                        nc.tensor.matmul(
                            bc[:, ci * MMCH:(ci + 1) * MMCH],
                            w2bc[:, o * C:(o + 1) * C],
                            f_sb[:, c0:c0 + MMCH],
                            start=True, stop=True,
                        )
                    nc.scalar.activation(
                        bcs[:, h2 * HB:(h2 + 1) * HB], bc[:],
                        mybir.ActivationFunctionType.Identity,
                        bias=b2bc[:, o:o + 1],
                    )
                # shifted x view: rows di..di+32, cols 3+dj..67+dj of the
                # guarded layout; odd dj reads the 1-shifted twin for alignment
                r0 = 3 + di
                c0 = 3 + dj
                if gp:
                    xv = xfr[:, r0:r0 + HH, c0:c0 + W]
                elif c0 % 2 == 0:
                    xv = xar[:, r0:r0 + HH, c0:c0 + W]
                else:
                    xv = xbr[:, r0:r0 + HH, c0 - 1:c0 - 1 + W]
                bcsr = bcs.rearrange("p (h w) -> p h w", w=W)
                if gp:
                    gprod = gppool.tile([C, P], f32, tag="gprod")
                    gprodr = gprod.rearrange("p (h w) -> p h w", w=W)
                    nc.gpsimd.tensor_mul(gprodr, xv, bcsr)
                    if o == GP_OFFS[0]:
                        nc.gpsimd.tensor_copy(acc_gp[:], gprod[:])
                    else:
                        nc.gpsimd.tensor_add(acc_gp[:], acc_gp[:], gprod[:])
                elif o == 0:
                    accr = acc.rearrange("p (h w) -> p h w", w=W)
                    nc.vector.tensor_mul(accr, xv, bcsr)
                elif o == 48:
                    if GP_OFFS:
                        nc.vector.tensor_add(acc[:], acc[:], acc_gp[:])
                    prod = ppool.tile([C, P], f16, tag="prod")
                    prodr = prod.rearrange("p (h w) -> p h w", w=W)
                    nc.vector.tensor_mul(prodr, xv, bcsr)
                    TH = P // 3 // 2 * 2
                    nc.vector.tensor_add(acc[:, :TH], acc[:, :TH], prod[:, :TH])
                    nc.sync.dma_start(out_d[:, :TH], acc[:, :TH])
                    nc.vector.tensor_add(acc[:, TH:2 * TH], acc[:, TH:2 * TH],
                                         prod[:, TH:2 * TH])
                    nc.scalar.dma_start(out_d[:, TH:2 * TH], acc[:, TH:2 * TH])
                    nc.vector.tensor_add(acc[:, 2 * TH:], acc[:, 2 * TH:],
                                         prod[:, 2 * TH:])
                    nc.gpsimd.dma_start(out_d[:, 2 * TH:], acc[:, 2 * TH:])
                else:
                    prod = ppool.tile([C, P], f16, tag="prod")
                    prodr = prod.rearrange("p (h w) -> p h w", w=W)
                    nc.vector.tensor_mul(prodr, xv, bcsr)
                    nc.vector.tensor_add(acc[:], acc[:], prod[:])

    nc.compile()
    return nc


def _get_nc():
    if "nc" not in _STATE:
        _STATE["nc"] = _build()
    return _STATE["nc"]


def _host_prep(x, w1, b1, bn_gamma, bn_beta, bn_mean, bn_var, w2, b2):
    x = np.asarray(x, dtype=np.float32)
    scale = np.asarray(bn_gamma) / np.sqrt(np.asarray(bn_var) + EPS)
    w1s = (np.asarray(w1) * scale[:, None]).astype(np.float32)
    b1f = (np.asarray(b1) * scale + np.asarray(bn_beta)
           - np.asarray(bn_mean) * scale).astype(np.float32)
    w1sT = np.ascontiguousarray(w1s.T).astype(np.float16)        # [128, 32]
    b1fc = np.ascontiguousarray(b1f[:, None])                    # [32, 1]
    w2f = np.asarray(w2, np.float32)                             # [49, 32]
    # W2BC[r, o*128 + c] = w2[o, r]
    w2bc = np.ascontiguousarray(
        np.broadcast_to(w2f.T[:, :, None], (32, 49, C)).reshape(32, 49 * C)
    ).astype(np.float16)
    b2bc = np.ascontiguousarray(
        np.broadcast_to(np.asarray(b2, np.float32), (C, 49))
    )

    x16 = x.astype(np.float16)
    in_maps = []
    for core in range(8):
        b, half = divmod(core, 2)
        h0 = HH * half
        xa = np.zeros((C, NXF), dtype=np.float16)
        lo = max(0, h0 - 3)
        hi = min(H, h0 + HH + 3)
        body = xa[:, XPAD:XPAD + NH * XROW].reshape(C, NH, XROW)
        body[:, lo - (h0 - 3):hi - (h0 - 3), 3:3 + W] = x16[b, :, lo:hi, :]
        in_maps.append({
            "xa": xa, "w1sT": w1sT, "b1f": b1fc,
            "w2bc": w2bc, "b2bc": b2bc,
        })
    return in_maps


def run(inputs: dict, trace: bool = False):
    from concourse.bass_utils import run_bass_kernel_spmd

    nc = _get_nc()
    in_maps = _host_prep(**inputs)
    res = run_bass_kernel_spmd(
        nc, in_maps, core_ids=list(range(8)), trace=trace,
    )
    out = np.zeros((B, C, H, W), dtype=np.float32)
    for core in range(8):
        b, half = divmod(core, 2)
        h0 = HH * half
        o = res.results[core]["out"].reshape(C, HH, W)
        out[b, :, h0:h0 + HH, :] = o.astype(np.float32)
    return out, res


def kernel(**inputs) -> np.ndarray:
    out, _ = run(inputs, trace=False)
    return out


# revision 18
# speedup vs baseline: 1.3855x; 1.3855x over previous
# Involution2d (K=7) Trainium2 kernel — 8-core SPMD, batch+spatial sharding.
#
# Sharding: 8 cores = (batch b in 0..3) x (H-half in 0..1). Each core computes
# a [128, 32, 64] output block.
#
# Per-core pipeline (fp16 data path, rel-err budget 2e-2):
#   1. f = relu(w1s^T @ xd + b1f)           TensorE + ScalarE   [32, 2048]
#   2. per offset o (49 total):
#      bc_o  = W2BC_o @ f                   TensorE (K=32)      [128, 2048] PSUM
#        (W2BC_o = w2 row o replicated into 128 columns -> the per-pixel
#         kernel value arrives already broadcast over channels; kerm is
#         never materialized and no per-offset DMA is needed)
#      bcs_o = bc_o + b2[o]                 ScalarE PSUM->SBUF fp16 (2 halves)
#      prod  = bcs_o * x_shift(o)           VectorE fp16 2x-mode
#      acc  += prod                         VectorE fp16 2x-mode
#
# x lives in a guarded stride-70 row layout (3 zero cols each side, 3 halo
# rows top/bottom) so every shifted read is exact zero padding — no mask.
# Shifted reads use 3D APs [128, 32, 64]; a twin copy of x offset by one
# element keeps odd-dj offsets 4-byte aligned (DVE 2x_1P mode requirement).
# acc/bcs/prod/f are dense [.., 2048] stride-1 tiles.
import numpy as np

EPS = 1e-5
KK = 7
C = 128
H = 64
W = 64
B = 4
HH = 32              # output rows per core
XROW = 70            # padded row stride: 3 | 64 | 3
NH = HH + 6          # rows incl. 3-row halos
XPAD = 4             # edge guard (even: preserves dj parity)
NXF = NH * XROW + 2 * XPAD   # 2668 x columns per core
QOFF = XPAD + 3 * XROW       # start of own rows in guarded coords (214)
P = HH * W           # 2048 dense output pixels
MMCH = 512           # matmul moving chunk (= PSUM bank, fp32)
GP_OFFS = ()   # GpSimd offload: tested (fp16 and fp32) — both slower

_STATE = {}


def _build():
    import concourse.tile as tile
    from concourse import bacc, mybir

    f32 = mybir.dt.float32
    f16 = mybir.dt.float16
    nc = bacc.Bacc("TRN2", target_bir_lowering=False, debug=False)

    xa_d = nc.dram_tensor("xa", [C, NXF], f16, kind="ExternalInput").ap()
    w1sT_d = nc.dram_tensor("w1sT", [C, 32], f16, kind="ExternalInput").ap()
    b1f_d = nc.dram_tensor("b1f", [32, 1], f32, kind="ExternalInput").ap()
    w2bc_d = nc.dram_tensor("w2bc", [32, 49 * C], f16, kind="ExternalInput").ap()
    b2bc_d = nc.dram_tensor("b2bc", [C, 49], f32, kind="ExternalInput").ap()
    out_d = nc.dram_tensor("out", [C, P], f16, kind="ExternalOutput").ap()

    with tile.TileContext(nc) as tc:
        with (
            tc.tile_pool(name="consts", bufs=1) as cpool,
            tc.tile_pool(name="bcs", bufs=4) as spool,
            tc.tile_pool(name="prod", bufs=3) as ppool,
            tc.tile_pool(name="gbcs", bufs=2) as gbpool,
            tc.tile_pool(name="gprod", bufs=2) as gppool,
            tc.tile_pool(name="pbc", bufs=4, space="PSUM") as pbc,
        ):
            # spread input DMAs across engines -> parallel hardware queues,
            # ordered so each consumer's gate arrives as late as it is needed
            HP = P // 2
            # xa: rows 0..18 on sync, rows 19..37 on scalar queue, first
            XSPL = XPAD + 19 * XROW
            xa = cpool.tile([C, NXF], f16, tag="xa")
            nc.sync.dma_start(xa[:, :XSPL], xa_d[:, :XSPL])
            nc.scalar.dma_start(xa[:, XSPL:], xa_d[:, XSPL:])
            w1sT = cpool.tile([C, 32], f16, tag="w1")
            nc.sync.dma_start(w1sT[:], w1sT_d)
            b1f = cpool.tile([32, 1], f32, tag="b1")
            nc.sync.dma_start(b1f[:], b1f_d)
            b2bc = cpool.tile([C, 49], f32, tag="b2bc")
            nc.sync.dma_start(b2bc[:], b2bc_d)
            w2bc = cpool.tile([32, 49 * C], f16, tag="w2bc")
            nc.scalar.dma_start(w2bc[:, :8 * C], w2bc_d[:, :8 * C])
            nc.gpsimd.dma_start(w2bc[:, 8 * C:], w2bc_d[:, 8 * C:])
            # twin of xa shifted by one element (keeps odd-dj reads 4B-aligned)
            xb = cpool.tile([C, NXF], f16, tag="xb")

            f_sb = cpool.tile([32, P], f16, tag="f")
            acc = cpool.tile([C, P], f16, tag="acc")
            if GP_OFFS:
                acc_gp = cpool.tile([C, P], f32, tag="accgp")
                xaf = cpool.tile([C, NXF], f32, tag="xaf")

            # guarded-layout shifted views of x (3D: [128, 32 rows, 64 w])
            xar = xa[:, XPAD:XPAD + NH * XROW].rearrange("p (h w) -> p h w", w=XROW)
            xbr = xb[:, XPAD:XPAD + NH * XROW].rearrange("p (h w) -> p h w", w=XROW)

            # ---- kernel-feature generation: f = relu(w1s^T @ xd + b1f) ----
            nc.gpsimd.tensor_copy(xb[:, :NXF - 1], xa[:, 1:])
            if GP_OFFS:
                nc.gpsimd.tensor_copy(xaf[:], xa[:])
                xfr = xaf[:, XPAD:XPAD + NH * XROW].rearrange(
                    "p (h w) -> p h w", w=XROW)
            for hg in range(2):
                f_ps = pbc.tile([32, P // 2], f32, tag="bc")
                for q in range(2):
                    r = 3 + 8 * (2 * hg + q)
                    xmv = xar[:, r:r + 8, 3:67]
                    nc.tensor.matmul(
                        f_ps[:, q * MMCH:(q + 1) * MMCH], w1sT[:],
                        xmv, start=True, stop=True,
                    )
                nc.scalar.activation(
                    f_sb[:, hg * HP:(hg + 1) * HP], f_ps[:],
                    mybir.ActivationFunctionType.Relu, bias=b1f[:],
                )

            # ---- involution accumulate over the 49 offsets ----
            HB = P // 2  # 1024: evac half (PSUM tile = 2 banks)
            for o in range(49):
                ip, jp = divmod(o, 7)
                di, dj = ip - 3, jp - 3
                gp = o in GP_OFFS
                if gp:
                    bcs = gbpool.tile([C, P], f32, tag="gbcs")
                else:
                    bcs = spool.tile([C, P], f16, tag="bcs")
                for h2 in range(2):
                    bc = pbc.tile([C, HB], f32, tag="bc")
                    for ci in range(HB // MMCH):
                        c0 = h2 * HB + ci * MMCH
                        nc.tensor.matmul(
                            bc[:, ci * MMCH:(ci + 1) * MMCH],
                            w2bc[:, o * C:(o + 1) * C],
                            f_sb[:, c0:c0 + MMCH],
                            start=True, stop=True,
                        )
                    nc.scalar.activation(
                        bcs[:, h2 * HB:(h2 + 1) * HB], bc[:],
                        mybir.ActivationFunctionType.Identity,
                        bias=b2bc[:, o:o + 1],
                    )
                # shifted x view: rows di..di+32, cols 3+dj..67+dj of the
                # guarded layout; odd dj reads the 1-shifted twin for alignment
                r0 = 3 + di
                c0 = 3 + dj
                if gp:
                    xv = xfr[:, r0:r0 + HH, c0:c0 + W]
                elif c0 % 2 == 0:
                    xv = xar[:, r0:r0 + HH, c0:c0 + W]
                else:
                    xv = xbr[:, r0:r0 + HH, c0 - 1:c0 - 1 + W]
                bcsr = bcs.rearrange("p (h w) -> p h w", w=W)
                if gp:
                    gprod = gppool.tile([C, P], f32, tag="gprod")
                    gprodr = gprod.rearrange("p (h w) -> p h w", w=W)
                    nc.gpsimd.tensor_mul(gprodr, xv, bcsr)
                    if o == GP_OFFS[0]:
                        nc.gpsimd.tensor_copy(acc_gp[:], gprod[:])
                    else:
                        nc.gpsimd.tensor_add(acc_gp[:], acc_gp[:], gprod[:])
                elif o == 0:
                    accr = acc.rearrange("p (h w) -> p h w", w=W)
                    nc.vector.tensor_mul(accr, xv, bcsr)
                elif o == 48:
                    if GP_OFFS:
                        nc.vector.tensor_add(acc[:], acc[:], acc_gp[:])
                    prod = ppool.tile([C, P], f16, tag="prod")
                    prodr = prod.rearrange("p (h w) -> p h w", w=W)
                    nc.vector.tensor_mul(prodr, xv, bcsr)
                    TH = P // 3 // 2 * 2
                    nc.vector.tensor_add(acc[:, :TH], acc[:, :TH], prod[:, :TH])
                    nc.sync.dma_start(out_d[:, :TH], acc[:, :TH])
                    nc.vector.tensor_add(acc[:, TH:2 * TH], acc[:, TH:2 * TH],
                                         prod[:, TH:2 * TH])
                    nc.scalar.dma_start(out_d[:, TH:2 * TH], acc[:, TH:2 * TH])
                    nc.vector.tensor_add(acc[:, 2 * TH:], acc[:, 2 * TH:],
                                         prod[:, 2 * TH:])
                    nc.gpsimd.dma_start(out_d[:, 2 * TH:], acc[:, 2 * TH:])
                else:
                    prod = ppool.tile([C, P], f16, tag="prod")
                    prodr = prod.rearrange("p (h w) -> p h w", w=W)
                    nc.vector.tensor_mul(prodr, xv, bcsr)
                    nc.vector.tensor_add(acc[:], acc[:], prod[:])

    nc.compile()
    return nc


def _get_nc():
    if "nc" not in _STATE:
        _STATE["nc"] = _build()
    return _STATE["nc"]


def _host_prep(x, w1, b1, bn_gamma, bn_beta, bn_mean, bn_var, w2, b2):
    x = np.asarray(x, dtype=np.float32)
    scale = np.asarray(bn_gamma) / np.sqrt(np.asarray(bn_var) + EPS)
    w1s = (np.asarray(w1) * scale[:, None]).astype(np.float32)
    b1f = (np.asarray(b1) * scale + np.asarray(bn_beta)
           - np.asarray(bn_mean) * scale).astype(np.float32)
    w1sT = np.ascontiguousarray(w1s.T).astype(np.float16)        # [128, 32]
    b1fc = np.ascontiguousarray(b1f[:, None])                    # [32, 1]
    w2f = np.asarray(w2, np.float32)                             # [49, 32]
    # W2BC[r, o*128 + c] = w2[o, r]
    w2bc = np.ascontiguousarray(
        np.broadcast_to(w2f.T[:, :, None], (32, 49, C)).reshape(32, 49 * C)
    ).astype(np.float16)
    b2bc = np.ascontiguousarray(
        np.broadcast_to(np.asarray(b2, np.float32), (C, 49))
    )

    x16 = x.astype(np.float16)
    in_maps = []
    for core in range(8):
        b, half = divmod(core, 2)
        h0 = HH * half
        xa = np.zeros((C, NXF), dtype=np.float16)
        lo = max(0, h0 - 3)
        hi = min(H, h0 + HH + 3)
        body = xa[:, XPAD:XPAD + NH * XROW].reshape(C, NH, XROW)
        body[:, lo - (h0 - 3):hi - (h0 - 3), 3:3 + W] = x16[b, :, lo:hi, :]
        in_maps.append({
            "xa": xa, "w1sT": w1sT, "b1f": b1fc,
            "w2bc": w2bc, "b2bc": b2bc,
        })
    return in_maps


def run(inputs: dict, trace: bool = False):
    from concourse.bass_utils import run_bass_kernel_spmd

    nc = _get_nc()
    in_maps = _host_prep(**inputs)
    res = run_bass_kernel_spmd(
        nc, in_maps, core_ids=list(range(8)), trace=trace,
    )
    out = np.zeros((B, C, H, W), dtype=np.float32)
    for core in range(8):
        b, half = divmod(core, 2)
        h0 = HH * half
        o = res.results[core]["out"].reshape(C, HH, W)
        out[b, :, h0:h0 + HH, :] = o.astype(np.float32)
    return out, res


def kernel(**inputs) -> np.ndarray:
    out, _ = run(inputs, trace=False)
    return out
